# revision 55
# baseline (speedup 1.0000x reference)
"""Trainium2 Bass kernel for nn_KuramotoHyperUniversal.

Data-parallel over batch across 8 NeuronCores (64 rows/core); weights
replicated. The (B,D,D) pairwise term is computed via the identity
  sum_j sin(y_j - y_i) A[i,j] = cos(y_i)*(A@sin(y))_i - sin(y_i)*(A@cos(y))_i
so it becomes two [64,512]x[512,512] matmuls instead of a 64MB tensor.

The kernel is HBM-bound on the replicated MLP weights, so:
  - weights ship in reduced precision via host-side casts: W0/W3 as
    bf16, W1/W2 (tanh-damped) as fp8-e4m3 scaled by 16, A^T as fp8
    scaled by 32 — rel err ~1.1e-2 vs the 2e-2 gate;
  - each weight matrix is host-packed chunk-major into its SBUF k-tile
    layout so a few large DMAs deliver whole output-chunk blocks (the
    shared HWDGE descriptor generator costs ~650ns per DMA, so count
    matters as much as bytes) and compute starts per chunk;
  - A is pre-transposed on the host (pure layout prep);
  - y/freqs/t and biases/tail-rows pack into two small aux DMAs.
Per chunk the PE accumulates 12 k-tile matmuls plus one combined
tail+bias matmul (stationary [hT_tail; ones], using an always-1.0
column carried in h through the transpose); tanh and the transposes of
chunk c are emitted after chunk c+1's matmuls so the PE never waits.
The t-column folds in as a (t-1)*ones bias matmul on layer 0. All
matmul inputs are bf16/fp8 stationary x bf16/fp8 moving into fp32
PSUM; the 1/16 fp8 scale comes out in the tanh activation's scale.
"""

import numpy as np
import ml_dtypes
from contextlib import ExitStack

import concourse.bass as bass
import concourse.mybir as mybir
import concourse.tile as tile
from concourse.vector_clock import ScopedClock, VectorClock
from concourse.bass_utils import run_bass_kernel_spmd
from concourse.masks import make_identity

DIM = 512
BATCH = 512
NCORES = 8
BS = BATCH // NCORES  # 64
H = 2 + 3 * DIM  # 1538
IN_SZ = 1 + 3 * DIM  # 1537
KH = 12  # full 128-row k-tiles in an H-sized contraction (plus 2 tail rows)
F32 = mybir.dt.float32
F32R = mybir.dt.float32r
BF16 = mybir.dt.bfloat16
FP8 = mybir.dt.float8e4
PI_HALF = float(np.pi / 2.0)

# W1/W2 ship as fp8-e4m3 scaled by MID_SCALE (tanh damps the ~1.1e-2
# quantization error; halves the two largest weight loads). Their bias
# and tail rows are pre-scaled on the host; the tanh activation applies
# 1/MID_SCALE. Set MID_FP8=False to fall back to bf16.
MID_FP8 = True
MID_SCALE = 16.0
A_SCALE = 32.0

# aux pack layouts (matmul and DVE operands all live on partition 0 to
# satisfy base-partition/lane-alignment rules):
# small-input packs (two DMAs total; matmul operands start at partition 0):
#   pk  [64, 1025] f32: [y[:, :512] | freqs | t column]
#   bpq [3, 5H] bf16, one H-column block per layer:
#     block 0: rows [W0[1024,:]; b0]      (stationary [(t-1)*ones; ones])
#     blocks 1-3: rows [W_l[1536]; W_l[1537]; b_l] (stationary [hT_tail; ones])
#   so each layer's ragged tail rows AND its bias land in ONE extra matmul
#   per chunk. t-1 and its broadcast run on the Activation engine so the
#   DVE queue head never waits on a late DMA.


def _split_drain_and_barrier(self, tick_clock, wait_clock):
    # Walrus in this container rejects >2 sync waits on one CTRL (drain)
    # instruction; emit one single-wait NOP per outstanding proc instead.
    gc = tick_clock.global_clock
    ticks = list(gc)
    for p, t in enumerate(ticks):
        if t > 0:
            v = [0] * len(ticks)
            v[p] = t
            nop = self.nc.sync.nop(nofuse=True, hint=f"drain_wait_{p}")
            wait_clock.add_sem_waits(nop.ins, ScopedClock({None: VectorClock(v)}))
    self.nc.sync.drain()
    self.nc.all_engine_barrier()
    popped = self.nc._tile_sem_poison_stack.pop()
    assert popped is self._sem_poison
    self.nc.clear_and_free_semaphores(list(self.sems.allocated().values()))
    self.nc.all_engine_barrier()


tile.TileContext._drain_and_barrier = _split_drain_and_barrier


def _r(ap):
    return ap.bitcast(F32R)


_MAX_WAITS = 1


def _split_waits(nc, limit=_MAX_WAITS):
    """Walrus rejects instructions carrying more than `limit` sync waits;
    move the excess onto same-engine NOPs inserted just before."""
    import bass_rust

    n = 0
    for f in nc.m.functions:
        for bb in f.blocks:
            out = []
            for inst in bb.instructions:
                si = inst.sync_info
                if si is not None and si.on_wait and len(si.on_wait) > limit:
                    waits = list(si.on_wait)
                    extra, keep = waits[:-limit], waits[-limit:]
                    for i in range(0, len(extra), limit):
                        nop = mybir.InstNoOp(name=f"I-wsplit-{n}", engine=inst.engine)
                        n += 1
                        nop.sync_info = bass_rust.SyncInfo(
                            on_wait=extra[i : i + limit], on_update=[]
                        )
                        out.append(nop)
                    inst.sync_info = bass_rust.SyncInfo(
                        on_wait=keep, on_update=list(si.on_update)
                    )
                out.append(inst)
            bb.instructions = out


def _build(reps=1, dma_only=False, split_queues=True, use_pool=True):
    nc = bass.Bass()
    AF = mybir.ActivationFunctionType

    pk_p = nc.declare_dram_parameter("pk", [BS, 2 * DIM + 1], F32, isOutput=False)
    bpq_p = nc.declare_dram_parameter("bpq", [3, 5 * H], BF16, isOutput=False)
    MDT = FP8 if MID_FP8 else BF16
    w_p = [
        nc.declare_dram_parameter("w0k", [128, KH * H], BF16, isOutput=False),
        nc.declare_dram_parameter("w1k", [128, KH * H], MDT, isOutput=False),
        nc.declare_dram_parameter("w2k", [128, KH * H], MDT, isOutput=False),
        nc.declare_dram_parameter("w3k", [128, KH * DIM], BF16, isOutput=False),
    ]
    at_p = nc.declare_dram_parameter("atk", [128, 4 * DIM], FP8, isOutput=False)
    out_p = nc.declare_dram_parameter("out", [BS, DIM + 1], F32, isOutput=True)

    with ExitStack() as ctx:
        tc = ctx.enter_context(tile.TileContext(nc))
        const = ctx.enter_context(tc.tile_pool(name="const", bufs=1))
        io = ctx.enter_context(tc.tile_pool(name="io", bufs=1))
        xtp = ctx.enter_context(tc.tile_pool(name="xtp", bufs=1))
        htp = ctx.enter_context(tc.tile_pool(name="htp", bufs=2))
        wp = ctx.enter_context(tc.tile_pool(name="wp", bufs=1))
        ps = ctx.enter_context(tc.tile_pool(name="ps", bufs=1, space="PSUM"))
        pst = ctx.enter_context(tc.tile_pool(name="pst", bufs=2, space="PSUM"))

        id64 = const.tile([64, 64], F32, tag="id64")
        make_identity(nc, id64[:])
        id64b = const.tile([64, 64], BF16, tag="id64b")
        nc.vector.tensor_copy(id64b[:], id64[:])
        ones_f = const.tile([1, 64], F32, tag="ones_f")
        nc.vector.memset(ones_f[:], 1.0)
        ones = const.tile([1, 64], BF16, tag="ones")
        nc.vector.tensor_copy(ones[:], ones_f[:])
        pih64 = const.tile([BS, 1], F32, tag="pih64")
        nc.vector.memset(pih64[:], PI_HALF)
        pih128 = const.tile([128, 1], F32, tag="pih128")
        nc.vector.memset(pih128[:], PI_HALF)
        neg1 = const.tile([1, 1], F32, tag="neg1")
        nc.vector.memset(neg1[:], -1.0)

        def _emit(rep):
            # ---- small inputs first, at the head of the Activation queue,
            # so the front compute (trig + transposes) starts immediately ----
            pk = io.tile([BS, 2 * DIM + 1], F32, tag="pk")
            nc.scalar.dma_start(out=pk[:], in_=pk_p[:])
            bpq = io.tile([3, 5 * H], BF16, tag="bpq")
            nc.scalar.dma_start(out=bpq[:], in_=bpq_p[:])
            yd = pk[:, 0:DIM]
            fr = pk[:, DIM : 2 * DIM]

            # ---- weight DMA stream, chunk-major packs (each sub-DMA
            # delivers whole output-chunk k-tile blocks so compute starts at
            # chunk granularity). Sub-DMAs spread across the available DGE
            # queues — on real hardware each queue's ring is drained by its
            # own DMA-engine set, so splitting the stream multiplies
            # effective HBM bandwidth (the pooled-engine cost model is
            # indifferent, the hardware is very much not). Bytes are balanced
            # greedily; W0 and A^T are pinned off the Activation ring, and
            # the Activation ring's share is emitted after the front trig
            # activations so they are never head-of-line blocked. ----
            engs = [nc.sync]
            if split_queues:
                engs.append(nc.scalar)
            if use_pool:
                engs.append(nc.gpsimd)

            ring_bytes = [0.0] * len(engs)
            act_idx = engs.index(nc.scalar) if split_queues else -1
            deferred_act = []

            def _wdma(t, p, a, b, nbytes, no_act=False):
                cand = [
                    i for i in range(len(engs)) if not (no_act and i == act_idx)
                ]
                i = min(cand, key=lambda j: ring_bytes[j])
                ring_bytes[i] += nbytes
                if i == act_idx:
                    deferred_act.append((t, p, a, b))
                else:
                    engs[i].dma_start(out=t[:, a:b], in_=p[:, a:b])

            def load_pack(name, p, n_cols, bounds, no_act=False):
                t = wp.tile([128, n_cols], p.dtype, tag=name, name=name)
                esz = mybir.dt.size(p.dtype)
                for a, b in zip(bounds[:-1], bounds[1:]):
                    _wdma(t, p, a, b, 128 * (b - a) * esz, no_act)
                return t

            # H-layer pack column layout (tail chunk first):
            #   [12*2 | 12*512 | 12*512 | 12*256 | 12*256]
            t0 = KH * 2
            hb = [0, t0 + KH * 512, t0 + KH * 1024, t0 + KH * 1280, KH * H]
            w0 = load_pack("w0", w_p[0], KH * H, [0, t0] + hb[1:], no_act=True)
            at = wp.tile([128, 4 * DIM], FP8, tag="at", name="at")
            _wdma(at, at_p, 0, 4 * DIM, 128 * 4 * DIM, no_act=True)
            w1 = load_pack("w1", w_p[1], KH * H, hb)
            w2 = load_pack("w2", w_p[2], KH * H, hb)
            w3 = load_pack("w3", w_p[3], KH * DIM,
                           [KH * j for j in range(0, 513, 128)])

            if dma_only:
                osb0 = io.tile([BS, DIM + 1], F32, tag="osb")
                nc.vector.memset(osb0[:], 0.0)
                nc.vector.tensor_scalar_add(osb0[:, 0:1], w0[0:64, 0:1], 0.0)
                nc.vector.tensor_scalar_add(osb0[:, 1:2], w1[0:64, 0:1], 0.0)
                nc.vector.tensor_scalar_add(osb0[:, 2:3], w2[0:64, 0:1], 0.0)
                nc.vector.tensor_scalar_add(osb0[:, 3:4], w3[0:64, 0:1], 0.0)
                nc.vector.tensor_scalar_add(osb0[:, 4:5], at[0:64, 0:1], 0.0)
                nc.scalar.dma_start(out=out_p[:], in_=osb0[:])
                return

            # C = cos(yd) = sin(yd + pi/2), S = sin(yd)   [64, 512] fp32
            C = io.tile([BS, DIM], F32, tag="C")
            nc.scalar.activation(C[:], yd, AF.Sin, bias=pih64[:])
            S = io.tile([BS, DIM], F32, tag="S")
            nc.scalar.activation(S[:], yd, AF.Sin)

            # ---- transposed layer-0 inputs (feature-on-partition, bf16) ----
            xC, xS, xF = [], [], []
            for j in range(4):
                p = pst.tile([128, 64], F32, tag="pstT", bufs=1)
                nc.tensor.transpose(p[:], yd[:, j * 128 : (j + 1) * 128], id64[:])
                c = xtp.tile([128, 64], BF16, tag=f"xC{j}")
                nc.scalar.activation(c[:], p[:], AF.Sin, bias=pih128[:])
                s = xtp.tile([128, 64], BF16, tag=f"xS{j}")
                nc.scalar.activation(s[:], p[:], AF.Sin)
                xC.append(c)
                xS.append(s)
            for j in range(4):
                p = pst.tile([128, 64], F32, tag="pstT", bufs=1)
                nc.tensor.transpose(p[:], fr[:, j * 128 : (j + 1) * 128], id64[:])
                f = xtp.tile([128, 64], BF16, tag=f"xF{j}")
                nc.vector.tensor_copy(f[:], p[:])
                xF.append(f)

            # ---- t-fold: build the [ (t-1)*ones ; ones ] stationary for
            # layer 0's combined tail+bias matmul. Partition-0-only ops:
            # r2 = [t-1, 1] row; ones^T @ r2 -> [64,2] psum; copy; transpose.
            tm1 = io.tile([1, 1], F32, tag="tm1")
            nc.scalar.activation(tm1[:], pk[0:1, 2 * DIM : 2 * DIM + 1],
                                 AF.Identity, bias=neg1[:])
            r2 = io.tile([1, 2], BF16, tag="r2")
            nc.scalar.activation(r2[:, 0:1], ones_f[:, 0:1], AF.Copy, scale=tm1[:])
            nc.scalar.activation(r2[:, 1:2], ones_f[:, 0:1], AF.Copy)
            p62 = pst.tile([BS, 2], F32, tag="pstT", bufs=1)
            nc.tensor.matmul(p62[:], ones[:], r2[:], start=True, stop=True)
            c62 = io.tile([BS, 2], BF16, tag="c62")
            nc.vector.tensor_copy(c62[:], p62[:])
            pt2 = pst.tile([2, 64], BF16, tag="pstTb")
            nc.tensor.transpose(pt2[:], c62[:], id64b[:])
            tsc = io.tile([2, 64], BF16, tag="tsc")
            nc.vector.tensor_copy(tsc[:], pt2[:])

            # The Activation ring's weight-stream share is drip-fed from
            # inside the layer-0/1 chunk loops (one dispatch per chunk, in
            # each chunk's activation psum-wait slack) — see mlp_layer.

            # ---- MLP: chunk-major, post-processing (tanh+transpose of
            # chunk c) emitted after chunk c+1's matmuls so PE never waits ----
            H_CHUNKS = [(1536, 2), (0, 512), (512, 512), (1024, 256), (1280, 256)]

            def mlp_layer(l, stat, tail_stat, wsb, chunks, tb_ap,
                          act_fn, h_tag, post, act_scale=1.0):
                # h carries one extra always-1.0 column so the tail transpose
                # yields [hT_tail; ones] directly (partition-0-based builds
                # only); layer 0 instead runs two 1-row bias matmuls. Hidden
                # layers store h as bf16 (identical precision to the bf16
                # stationary tiles) so their PE transposes run 1 cycle/row.
                out_dim = max(o + n for o, n in chunks)
                hdt = BF16 if act_fn == AF.Tanh else F32
                h = io.tile([BS, out_dim + 1], hdt, tag=h_tag)
                if act_fn == AF.Tanh:
                    nc.vector.memset(h[:, out_dim : out_dim + 1], 1.0)
                col = 0
                pend = None
                for o, n in chunks:
                    # PSUM tiles round to whole banks; reuse the three
                    # 512-wide bank tags for the tail and layer-3 chunks.
                    ptag = {
                        (1536, 2): "psC",
                        (0, 512): "psA",
                        (512, 512): "psB",
                        (1024, 256): "psC",
                        (1280, 256): "psD",
                        (0, 256): "psA",
                        (256, 256): "psB",
                        (0, 128): "psA",
                        (128, 128): "psB",
                        (256, 128): "psC",
                        (384, 128): "psD",
                    }[(o, n)]
                    psum = ps.tile([BS, n], F32, tag=ptag)
                    for k in range(KH):
                        nc.tensor.matmul(
                            psum[:],
                            stat[k][:],
                            wsb[:, col + k * n : col + (k + 1) * n],
                            start=(k == 0),
                            stop=False,
                        )
                    nc.tensor.matmul(
                        psum[:], tail_stat[:], tb_ap[:, o : o + n],
                        start=False, stop=True,
                    )
                    nc.scalar.activation(h[:, o : o + n], psum[:], act_fn,
                                         scale=act_scale)
                    if deferred_act:
                        t, p, a, b = deferred_act.pop(0)
                        nc.scalar.dma_start(out=t[:, a:b], in_=p[:, a:b])
                    if pend is not None:
                        post(h, *pend)
                    pend = (o, n)
                    col += KH * n
                post(h, *pend)
                return h

            def make_transpose_post(store):
                def post(h, o, n):
                    if n == 2:
                        p2 = pst.tile([3, 64], BF16, tag="pstTb")
                        nc.tensor.transpose(p2[:], h[:, o : o + 3], id64b[:])
                        ht2 = htp.tile([3, 64], BF16, tag="hTtail")
                        nc.vector.tensor_copy(ht2[:], p2[:])
                        store["tail"] = ht2
                        return
                    for j in range(o // 128, (o + n) // 128):
                        p = pst.tile([128, 64], BF16, tag="pstTb")
                        nc.tensor.transpose(
                            p[:], h[:, j * 128 : (j + 1) * 128], id64b[:]
                        )
                        ht = htp.tile([128, 64], BF16, tag=f"hT{j}")
                        nc.vector.tensor_copy(ht[:], p[:])
                        store[j] = ht
                return post

            st = {}
            mlp_layer(0, xC + xS + xF, tsc, w0, H_CHUNKS,
                      bpq[0:2, 0:H],
                      AF.Tanh, "h0", make_transpose_post(st))
            for l, wsb in ((1, w1), (2, w2)):
                stat = [st[j] for j in range(KH)]
                tail = st["tail"]
                st = {}
                mlp_layer(l, stat, tail, wsb, H_CHUNKS,
                          bpq[0:3, l * H : (l + 1) * H],
                          AF.Tanh, f"h{l % 2}", make_transpose_post(st),
                          act_scale=(1.0 / MID_SCALE) if MID_FP8 else 1.0)
                if l == 1:
                    # ---- forcesum = (C*(S@A^T) - S*(C@A^T)) / DIM ----
                    # emitted here so PE reaches these matmuls after A^T's
                    # late slot in the DMA stream; fs is only needed by the
                    # output chain.
                    fs = io.tile([BS, DIM], F32, tag="fs")
                    for name, xt in (("AS", xS), ("AC", xC)):
                        ptr = ps.tile([BS, DIM], F32, tag="pstrig")
                        for j in range(4):
                            nc.tensor.matmul(
                                ptr[:],
                                xt[j][:],
                                at[:, j * DIM : (j + 1) * DIM],
                                start=(j == 0),
                                stop=(j == 3),
                            )
                        if name == "AS":
                            nc.vector.tensor_mul(fs[:], C[:], ptr[:])
                        else:
                            tmp = io.tile([BS, DIM], F32, tag="fs2")
                            nc.vector.tensor_mul(tmp[:], S[:], ptr[:])
                            nc.vector.tensor_sub(fs[:], fs[:], tmp[:])
                    nc.vector.tensor_scalar_mul(fs[:], fs[:], 1.0 / (DIM * A_SCALE))

            # ---- layer 3 + output chain per 128-col chunk ----
            out_sb = io.tile([BS, DIM + 1], F32, tag="osb")
            fm = io.tile([BS, DIM], F32, tag="fm")
            sq = io.tile([BS, DIM], F32, tag="sq")
            f1p = io.tile([BS, 4], F32, tag="f1p")

            def out_post(cf, o, n):
                nc.vector.tensor_mul(fm[:, o : o + n], cf[:, o : o + n], fs[:, o : o + n])
                nc.vector.tensor_add(
                    out_sb[:, o : o + n], fm[:, o : o + n], fr[:, o : o + n]
                )
                nc.scalar.activation(
                    sq[:, o : o + n], cf[:, o : o + n], AF.Square,
                    accum_out=f1p[:, o // 128 : o // 128 + 1],
                )

            stat = [st[j] for j in range(KH)]
            cforce = mlp_layer(3, stat, st["tail"], w3,
                               [(j * 128, 128) for j in range(4)],
                               bpq[0:3, 3 * H : 3 * H + DIM],
                               AF.Copy, "h1", out_post)
            nc.sync.dma_start(out=out_p[:, 0:256], in_=out_sb[:, 0:256])
            # f1 = sum of the four per-chunk partials (free-dim reduce)
            nc.scalar.activation(
                sq[:, 0:4], f1p[:], AF.Copy,
                accum_out=out_sb[:, DIM : DIM + 1],
            )
            nc.sync.dma_start(out=out_p[:, 256 : DIM + 1], in_=out_sb[:, 256 : DIM + 1])

        for _rep in range(reps):
            _emit(_rep)

    _split_waits(nc)
    return nc


_NC_CACHE = {}

_BF = ml_dtypes.bfloat16


_F8 = np.dtype(ml_dtypes.float8_e4m3)


def _pack_ktiles(W, row_starts, chunks, dtype=_BF):
    """Chunk-major k-tile pack: for each output-column chunk (o, n), the 12
    k-tiles' [128, n] blocks are laid out contiguously, so one sub-DMA
    delivers everything a chunk's accumulation needs."""
    blocks = []
    for o, n in chunks:
        t = np.stack([W[r : r + 128, o : o + n] for r in row_starts], axis=1)
        blocks.append(t.reshape(128, len(row_starts) * n))
    return np.ascontiguousarray(np.concatenate(blocks, axis=1)).astype(dtype)


def _prep(inputs):
    W0, W1, W2, W3 = (np.asarray(inputs[f"W{i}"], np.float32) for i in range(4))
    A = np.asarray(inputs["A"], np.float32)
    t = np.asarray(inputs["t"], np.float32)

    hch = [(1536, 2), (0, 512), (512, 512), (1024, 256), (1280, 256)]
    w0_rows = list(range(0, 1024, 128)) + list(range(1025, 1537, 128))
    shared = {
        "w0k": _pack_ktiles(W0, w0_rows, hch),
        "w1k": _pack_ktiles(W1 * MID_SCALE if MID_FP8 else W1,
                            range(0, 1536, 128), hch,
                            dtype=_F8 if MID_FP8 else _BF),
        "w2k": _pack_ktiles(W2 * MID_SCALE if MID_FP8 else W2,
                            range(0, 1536, 128), hch,
                            dtype=_F8 if MID_FP8 else _BF),
        "w3k": _pack_ktiles(W3, range(0, 1536, 128), [(j * 128, 128) for j in range(4)]),
        "atk": _pack_ktiles(np.ascontiguousarray(A.T) * A_SCALE,
                            range(0, 512, 128), [(0, 512)], dtype=_F8),
    }
    b = [np.asarray(inputs[f"b{i}"], np.float32) for i in range(4)]
    ms = MID_SCALE if MID_FP8 else 1.0
    bpq = np.zeros((3, 5 * H), np.float32)
    bpq[0, 0:H] = W0[1024]
    bpq[1, 0:H] = b[0]
    bpq[0:2, H : 2 * H] = W1[1536:1538] * ms
    bpq[2, H : 2 * H] = b[1] * ms
    bpq[0:2, 2 * H : 3 * H] = W2[1536:1538] * ms
    bpq[2, 2 * H : 3 * H] = b[2] * ms
    bpq[0:2, 3 * H : 3 * H + DIM] = W3[1536:1538]
    bpq[2, 3 * H : 3 * H + DIM] = b[3]
    shared["bpq"] = bpq.astype(_BF)
    shared["t"] = t
    return shared


def _in_maps(inputs):
    shared = _prep(inputs)
    t = shared.pop("t")
    y = np.asarray(inputs["y"], dtype=np.float32)
    freqs = np.asarray(inputs["freqs"], dtype=np.float32)
    in_maps = []
    for i in range(NCORES):
        m = dict(shared)
        pk = np.empty((BS, 2 * DIM + 1), np.float32)
        pk[:, 0:DIM] = y[i * BS : (i + 1) * BS, 0:DIM]
        pk[:, DIM : 2 * DIM] = freqs[i * BS : (i + 1) * BS]
        pk[:, 2 * DIM] = t[0]
        m["pk"] = pk
        in_maps.append(m)
    return in_maps


def kernel(**inputs):
    key = "nc"
    if key not in _NC_CACHE:
        _NC_CACHE[key] = _build()
    nc = _NC_CACHE[key]

    res = run_bass_kernel_spmd(nc, _in_maps(inputs), core_ids=list(range(NCORES)))
    out = np.concatenate([res.results[i]["out"] for i in range(NCORES)], axis=0)
    return out.astype(np.float32)
